# revision 17
# baseline (speedup 1.0000x reference)
"""Trainium2 Bass kernel for Transformer-XL style relative-position MHSA.

Problem: nn_MultiHeadSelfAttention_14989435863450
  B=2, S=2048, D=512, H=8, dh=64, fp32 I/O.

Sharding (8 cores): core c -> batch b = c//4, head pair h0 = 2*(c%4).
Each core computes its 2 heads' attention and the partial output
projection (out_slice @ Wo[slice]); host sums 4 partials per batch and
adds the constant (bv @ Wo + bo) row vector.

Math folds (exact):
  - bq folds into u,v:  u_eff = (u + bq) / sqrt(D)
  - bk adds a per-query-row constant to scores -> cancels in softmax
  - bv contributes attn-weighted 1 * bv = bv -> host-side constant
  - 1/sqrt(D) folded into q at evacuation time

Relative shift: pos scores are streamed to a DRAM buffer PB[S, S+1]
with rows [0 | posrow_i]; reading PB.flat[S : S + S*S] as [S, S] is
exactly Transformer-XL's pad-reshape-slice shift (including the wrap).
"""

import math
from contextlib import ExitStack

import ml_dtypes
import numpy as np

BF16NP = ml_dtypes.bfloat16

import concourse.bass as bass
import concourse.bacc as bacc_mod
import concourse.mybir as mybir
import concourse.tile as tile
from concourse.bass import ts, ds
from concourse.bass_utils import run_bass_kernel_spmd
from concourse.masks import make_identity

FP32 = mybir.dt.float32
F32R = mybir.dt.float32r
BF16 = mybir.dt.bfloat16

D_MODEL = 512
NUM_HEADS = 8
D_HEAD = 64
DH2 = 2 * D_HEAD  # head-pair width per core
B_FULL = 2
S_FULL = 2048
P = 128
ISQ = 1.0 / math.sqrt(D_MODEL)

Exp = mybir.ActivationFunctionType.Exp
ADD = mybir.AluOpType.add
MULT = mybir.AluOpType.mult


def build_nc(S=S_FULL, use_f32r=False, reps=1, accum_shift=False, transpose_ring='sp', shift_ring='sp', pb_ring='sp', out_ring='sp', fake_transpose=False, pe_transpose=True):
    """Build the single-core Bass program (SPMD: same program, 8 cores)."""
    nc = bacc_mod.Bacc()
    NB = S // P          # query blocks
    NK = S // P          # key tiles
    CH = min(512, S)     # score column chunk (PSUM bank)
    NCH = S // CH        # chunks per row
    KD = D_MODEL // P    # contraction tiles over D

    MMDT = F32R if use_f32r else BF16

    xT = nc.declare_dram_parameter("xT", [D_MODEL, S], MMDT, isOutput=False)
    posT = nc.declare_dram_parameter("posT", [D_MODEL, S], MMDT, isOutput=False)
    Wq = nc.declare_dram_parameter("Wq", [D_MODEL, DH2], MMDT, isOutput=False)
    Wk = nc.declare_dram_parameter("Wk", [D_MODEL, DH2], MMDT, isOutput=False)
    Wv = nc.declare_dram_parameter("Wv", [D_MODEL, DH2], MMDT, isOutput=False)
    Wp = nc.declare_dram_parameter("Wp", [D_MODEL, DH2], MMDT, isOutput=False)
    Wo2 = nc.declare_dram_parameter("Wo2", [2, D_HEAD, D_MODEL], MMDT, isOutput=False)
    ueff = nc.declare_dram_parameter("ueff", [DH2, 1], FP32, isOutput=False)
    veff = nc.declare_dram_parameter("veff", [DH2, 1], FP32, isOutput=False)
    out_partial = nc.declare_dram_parameter("out_partial", [S, D_MODEL], FP32, isOutput=True)

    with ExitStack() as ctx:
        tc = ctx.enter_context(tile.TileContext(nc))
        consts = ctx.enter_context(tc.tile_pool(name="consts", bufs=1))
        blk = ctx.enter_context(tc.tile_pool(name="blk", bufs=3))
        dram = ctx.enter_context(tc.tile_pool(name="dram", bufs=1, space="DRAM"))
        psA = ctx.enter_context(tc.tile_pool(name="psA", bufs=1, space="PSUM"))
        psB = ctx.enter_context(tc.tile_pool(name="psB", bufs=1 if pe_transpose else 2, space="PSUM"))
        psE = ctx.enter_context(tc.tile_pool(name="psE", bufs=1, space="PSUM")) if pe_transpose else None
        psC = ctx.enter_context(tc.tile_pool(name="psC", bufs=1, space="PSUM"))
        psD = ctx.enter_context(tc.tile_pool(name="psD", bufs=1, space="PSUM"))

        # ---- load constants / inputs ----
        xT_sb = consts.tile([P, KD, S], MMDT)
        nc.sync.dma_start(xT_sb[:], xT.rearrange("(o p) s -> p o s", p=P))
        posT_sb = consts.tile([P, KD, S], MMDT)
        nc.sync.dma_start(posT_sb[:], posT.rearrange("(o p) s -> p o s", p=P))
        w_sbs = {}
        for nm, handle in (("Wq", Wq), ("Wk", Wk), ("Wv", Wv), ("Wp", Wp)):
            w_sb = consts.tile([P, KD, DH2], MMDT, name=f"{nm}_sb")
            nc.sync.dma_start(w_sb[:], handle.rearrange("(o p) m -> p o m", p=P))
            w_sbs[nm] = w_sb
        Wo_sb = consts.tile([D_HEAD, 2, D_MODEL], MMDT)
        nc.sync.dma_start(Wo_sb[:], Wo2.rearrange("t p n -> p t n"))
        ident_sb = consts.tile([P, P], BF16)
        make_identity(nc, ident_sb[:])
        ueff_sb = consts.tile([DH2, 1], FP32)
        nc.sync.dma_start(ueff_sb[:], ueff[:, :])
        veff_sb = consts.tile([DH2, 1], FP32)
        nc.sync.dma_start(veff_sb[:], veff[:, :])

        # ---- projections ----
        # qT/kT/pT: [DH2, S] = W.T @ x  (lhsT = W [D, DH2], rhs = xT [D, S])
        def proj_T(w_sb, src_sb):
            pq = psA.tile([P, S], FP32, tag="psA", name="pq")
            for chn in range(NCH):
                for kt in range(KD):
                    nc.tensor.matmul(
                        pq[:, ts(chn, CH)],
                        lhsT=w_sb[:, kt, :],
                        rhs=src_sb[:, kt, ts(chn, CH)],
                        start=(kt == 0),
                        stop=(kt == KD - 1),
                    )
            return pq

        qTu = consts.tile([DH2, S], MMDT)
        qTv = consts.tile([DH2, S], MMDT)
        pq = proj_T(w_sbs["Wq"], xT_sb)
        nc.vector.tensor_scalar(qTu[:], pq[:], ISQ, ueff_sb[:, 0:1], MULT, ADD)
        nc.vector.tensor_scalar(qTv[:], pq[:], ISQ, veff_sb[:, 0:1], MULT, ADD)
        kT = consts.tile([DH2, S], MMDT)
        pk = proj_T(w_sbs["Wk"], xT_sb)
        nc.scalar.copy(kT[:], pk[:])
        pT = consts.tile([DH2, S], MMDT)
        pp_ = proj_T(w_sbs["Wp"], posT_sb)
        nc.scalar.copy(pT[:], pp_[:])

        # vv: [S, DH2] natural (bf16 for the attn@v matmuls)
        vv_sb = consts.tile([P, NK, DH2], BF16)
        for st in range(NK):
            pv = psB.tile([P, CH], FP32, tag="psB", name="pv")
            for kt in range(KD):
                nc.tensor.matmul(
                    pv[:, :DH2],
                    lhsT=xT_sb[:, kt, ts(st, P)],
                    rhs=w_sbs["Wv"][:, kt, :],
                    start=(kt == 0),
                    stop=(kt == KD - 1),
                )
            nc.any.tensor_copy(vv_sb[:, st, :], pv[:, :DH2])

        # ---- per-head DRAM pos-score buffers (padded for the rel-shift) ----
        PB = [
            dram.tile([S, S + 1], BF16, name=f"pb{h}")
            for h in range(2)
        ]

        def pos_block(h, ib):
            """pos scores for q rows [128*ib, 128*ib+128) -> PB[h] rows."""
            pe = blk.tile([P, S + 1], BF16, tag="posext", name="pe")
            nc.vector.memset(pe[:, 0:1], 0.0)
            for chn in range(NCH):
                pp = psB.tile([P, CH], FP32, tag="psB", name="pp")
                nc.tensor.matmul(
                    pp[:],
                    lhsT=qTv[ds(h * D_HEAD, D_HEAD), ts(ib, P)],
                    rhs=pT[ds(h * D_HEAD, D_HEAD), ts(chn, CH)],
                    start=True,
                    stop=True,
                )
                if chn % 2 == 0:
                    nc.scalar.copy(pe[:, 1 + chn * CH : 1 + (chn + 1) * CH], pp[:])
                else:
                    nc.vector.tensor_copy(pe[:, 1 + chn * CH : 1 + (chn + 1) * CH], pp[:])
            (nc.scalar if pb_ring == 'act' else nc.sync).dma_start(PB[h][ts(ib, P), :], pe[:])

        def head_block(h, ib):
            """content + shifted pos + softmax + transpose for one q block."""
            # content matmuls -> PSUM -> evac (bf16), then the shifted pos
            # read is an SWDGE accumulate-DMA straight onto the content tile.
            pc = psA.tile([P, S], FP32, tag="psA", name="pc")
            for chn in range(NCH):
                nc.tensor.matmul(
                    pc[:, ts(chn, CH)],
                    lhsT=qTu[ds(h * D_HEAD, D_HEAD), ts(ib, P)],
                    rhs=kT[ds(h * D_HEAD, D_HEAD), ts(chn, CH)],
                    start=True,
                    stop=True,
                )
            sc = blk.tile([P, S], BF16, tag="scores", name="sc")
            flat = PB[h].flatten()
            view = flat[ds(S + ib * P * S, P * S)].rearrange("(p s) -> p s", s=S)
            if accum_shift:
                nc.vector.tensor_copy(sc[:], pc[:])
                nc.gpsimd.dma_start(sc[:], view, accum_op=ADD)
            else:
                sh = blk.tile([P, S], BF16, tag="shift", name="sh")
                if shift_ring == "pool":
                    nc.gpsimd.dma_start(sh[:], view)
                else:
                    nc.sync.dma_start(sh[:], view)
                nc.vector.tensor_tensor(sc[:], pc[:], sh[:], ADD)
            at = blk.tile([P, S], BF16, tag="attn", name="at")
            sums = blk.tile([P, 1], FP32, tag="sums", name="sums")
            nc.scalar.activation(at[:], sc[:], Exp, accum_out=sums[:])
            rec = blk.tile([P, 1], FP32, tag="rec", name="rec")
            nc.vector.reciprocal(rec[:], sums[:])
            nc.vector.tensor_scalar_mul(at[:], at[:], rec[:, 0:1])
            atT = blk.tile([P, NK, P], BF16, tag=f"attnT{h}", name="atT")
            if pe_transpose:
                G = min(4, NK)
                for g in range(NK // G):
                    pt4 = psE.tile([P, G, P], BF16, tag="psE", name="pt4")
                    for j in range(G):
                        kt = G * g + j
                        nc.tensor.transpose(pt4[:, j, :], at[:, ts(kt, P)], ident_sb[:])
                    if g % 2 == 0:
                        nc.vector.tensor_copy(atT[:, ts(g, G), :], pt4[:])
                    else:
                        nc.scalar.copy(atT[:, ts(g, G), :], pt4[:])
            elif fake_transpose:
                nc.sync.dma_start(atT[:], at[:].rearrange("p (a b) -> p a b", b=P))
            else:
                (nc.scalar if transpose_ring == 'act' else nc.sync).dma_start_transpose(atT[:], at[:])
            return atT

        def out_block(ib, atTs):
            o2s = []
            for h in range(2):
                po = psC.tile([D_HEAD, P], FP32, tag="psC", name="po")
                for kt in range(NK):
                    nc.tensor.matmul(
                        po[:],
                        lhsT=vv_sb[:, kt, ds(h * D_HEAD, D_HEAD)],
                        rhs=atTs[h][:, kt, :],
                        start=(kt == 0),
                        stop=(kt == NK - 1),
                    )
                o2 = blk.tile([D_HEAD, P], MMDT, tag=f"o2_{h}", name="o2")
                nc.scalar.copy(o2[:], po[:])
                o2s.append(o2)
            pw = psD.tile([P, D_MODEL], FP32, tag="psD", name="pw")
            for h in range(2):
                nc.tensor.matmul(
                    pw[:],
                    lhsT=o2s[h][:],
                    rhs=Wo_sb[:, h, :],
                    start=(h == 0),
                    stop=(h == 1),
                )
            fin = blk.tile([P, D_MODEL], FP32, tag="fin", name="fin")
            nc.vector.tensor_copy(fin[:], pw[:])
            (nc.scalar if out_ring == 'act' else nc.sync).dma_start(out_partial[ts(ib, P), :], fin[:])

        # ---- main pipeline ----
        def pipeline():
            for h in range(2):
                pos_block(h, 0)
            for ib in range(NB):
                atTs = []
                for h in range(2):
                    if ib + 1 < NB:
                        pos_block(h, ib + 1)
                    atTs.append(head_block(h, ib))
                out_block(ib, atTs)

        if reps == 1:
            pipeline()
        else:
            with tc.For_i(0, reps, 1):
                pipeline()

    nc.finalize()
    return nc


FP8 = mybir.dt.float8e4


def build_nc2(
    S=S_FULL,
    pb_dt=BF16,
    posev="vvvvvvvv",  # evac engine per (chn,h) slot: v=vector, s=scalar
    sh_ring="sp",
    pb_ring="act",
    xbar_ring="sp",
    out_ring="act",
):
    """v2: exp-from-PSUM via identity-accum, row-tiled scores, [q,dh] attn@v
    with norm folded into the per-partition o3 evac, merged out-proj,
    software-pipelined pos(+2)/sh(+1)/content(ib)/output(ib-1)."""
    nc = bacc_mod.Bacc()
    NB = S // P
    NK = S // P
    CH = 512
    NCH = S // CH
    CH2 = 1024
    KD = D_MODEL // P

    xT = nc.declare_dram_parameter("xT", [D_MODEL, S], BF16, isOutput=False)
    posT = nc.declare_dram_parameter("posT", [D_MODEL, S], BF16, isOutput=False)
    Wq = nc.declare_dram_parameter("Wq", [D_MODEL, DH2], BF16, isOutput=False)
    Wk = nc.declare_dram_parameter("Wk", [D_MODEL, DH2], BF16, isOutput=False)
    Wv = nc.declare_dram_parameter("Wv", [D_MODEL, DH2], BF16, isOutput=False)
    Wp = nc.declare_dram_parameter("Wp", [D_MODEL, DH2], BF16, isOutput=False)
    Wo128 = nc.declare_dram_parameter("Wo128", [DH2, D_MODEL], BF16, isOutput=False)
    ueff = nc.declare_dram_parameter("ueff", [DH2, 1], FP32, isOutput=False)
    veff = nc.declare_dram_parameter("veff", [DH2, 1], FP32, isOutput=False)
    out_partial = nc.declare_dram_parameter("out_partial", [S, D_MODEL], BF16, isOutput=True)

    def ring(which):
        return nc.scalar if which == "act" else nc.sync

    with ExitStack() as ctx:
        tc = ctx.enter_context(tile.TileContext(nc))
        consts = ctx.enter_context(tc.tile_pool(name="consts", bufs=1))
        blk = ctx.enter_context(tc.tile_pool(name="blk", bufs=2))
        dram = ctx.enter_context(tc.tile_pool(name="dram", bufs=1, space="DRAM"))
        ps = ctx.enter_context(tc.tile_pool(name="ps", bufs=1, space="PSUM"))

        # ---- load constants / inputs ----
        xT_sb = consts.tile([P, KD, S], BF16)
        nc.sync.dma_start(xT_sb[:], xT.rearrange("(o p) s -> p o s", p=P))
        posT_sb = consts.tile([P, KD, S], BF16)
        nc.sync.dma_start(posT_sb[:], posT.rearrange("(o p) s -> p o s", p=P))
        w_sbs = {}
        for nm, handle in (("Wq", Wq), ("Wk", Wk), ("Wv", Wv), ("Wp", Wp)):
            w_sb = consts.tile([P, KD, DH2], BF16, name=f"{nm}_sb")
            nc.sync.dma_start(w_sb[:], handle.rearrange("(o p) m -> p o m", p=P))
            w_sbs[nm] = w_sb
        Wo_sb = consts.tile([DH2, D_MODEL], BF16)
        nc.sync.dma_start(Wo_sb[:], Wo128[:, :])
        identsh = consts.tile([P, P], pb_dt)
        make_identity(nc, identsh[:])
        identb = consts.tile([P, P], BF16)
        make_identity(nc, identb[:])
        ueff_sb = consts.tile([DH2, 1], FP32)
        nc.sync.dma_start(ueff_sb[:], ueff[:, :])
        veff_sb = consts.tile([DH2, 1], FP32)
        nc.sync.dma_start(veff_sb[:], veff[:, :])

        qTu = consts.tile([DH2, S], BF16)
        qTv = consts.tile([DH2, S], BF16)
        kT = consts.tile([DH2, S], BF16)
        pT = consts.tile([DH2, S], BF16)

        # ---- projections (reuse steady-state PSUM tags) ----
        def proj_T(w_sb, src_sb, dsts):
            for half in range(S // CH2):
                pq = ps.tile([P, CH2], FP32, tag="pc", bufs=2, name="pq")
                for seg in range(CH2 // CH):
                    for kt in range(KD):
                        nc.tensor.matmul(
                            pq[:, ts(seg, CH)],
                            lhsT=w_sb[:, kt, :],
                            rhs=src_sb[:, kt, ds(half * CH2 + seg * CH, CH)],
                            start=(kt == 0),
                            stop=(kt == KD - 1),
                        )
                for dst, scale, bias in dsts:
                    if scale is None:
                        nc.scalar.copy(dst[:, ts(half, CH2)], pq[:])
                    else:
                        nc.vector.tensor_scalar(
                            dst[:, ts(half, CH2)], pq[:], scale, bias, MULT, ADD
                        )

        proj_T(w_sbs["Wq"], xT_sb, [(qTu, ISQ, ueff_sb[:, 0:1]), (qTv, ISQ, veff_sb[:, 0:1])])
        proj_T(w_sbs["Wk"], xT_sb, [(kT, None, None)])
        proj_T(w_sbs["Wp"], posT_sb, [(pT, None, None)])

        vv_sb = consts.tile([P, NK, DH2], BF16)
        for st in range(NK):
            pv = ps.tile([P, DH2], FP32, tag=("po" if st % 2 == 0 else "pw"), bufs=1, name="pv")
            for kt in range(KD):
                nc.tensor.matmul(
                    pv[:],
                    lhsT=xT_sb[:, kt, ts(st, P)],
                    rhs=w_sbs["Wv"][:, kt, :],
                    start=(kt == 0),
                    stop=(kt == KD - 1),
                )
            nc.any.tensor_copy(vv_sb[:, st, :], pv[:])

        # ---- per-head DRAM pos-score buffers, rows [0 | posrow] ----
        PB = [dram.tile([S, S + 1], pb_dt, name=f"pb{h}") for h in range(2)]

        def new_frags():
            fr = []
            for h in range(2):
                f = blk.tile([P, S + 1], pb_dt, tag=f"frag{h}", name=f"frag{h}")
                nc.gpsimd.memset(f[:, 0:1], 0.0)
                fr.append(f)
            return fr

        def pos_chunks(ib, chns, frags):
            for chn in chns:
                for h in range(2):
                    pp = ps.tile([P, CH], FP32, tag="pp", bufs=2, name="pp")
                    nc.tensor.matmul(
                        pp[:],
                        lhsT=qTv[ds(h * D_HEAD, D_HEAD), ts(ib, P)],
                        rhs=pT[ds(h * D_HEAD, D_HEAD), ts(chn, CH)],
                        start=True,
                        stop=True,
                    )
                    dst = frags[h][:, 1 + chn * CH : 1 + (chn + 1) * CH]
                    if posev[chn * 2 + h] == "v":
                        nc.vector.tensor_copy(dst, pp[:])
                    else:
                        nc.scalar.copy(dst, pp[:])

        def pb_write(ib, frags):
            for h in range(2):
                ring(pb_ring).dma_start(PB[h][ts(ib, P), :], frags[h][:])

        def sh_read(ib):
            out = []
            for h in range(2):
                shh = blk.tile([P, S], pb_dt, tag=f"sh{h}", name=f"sh{h}")
                flat = PB[h].flatten()
                view = flat[ds(S + ib * P * S, P * S)].rearrange("(p s) -> p s", s=S)
                for half in range(2):
                    cols = ds(half * (S // 2), S // 2)
                    ring(sh_ring).dma_start(shh[:, cols], view[:, cols])
                out.append(shh)
            return out

        def out_attnv(ib, atT2, recs):
            # single PSUM bank, ONE start group: only the very first MM has
            # start=True (a start resets the whole bank)
            po = ps.tile([P, DH2], FP32, tag="po", name="po")
            for kt in range(NK):
                for h in range(2):
                    nc.tensor.matmul(
                        po[:, ds(h * D_HEAD, D_HEAD)],
                        lhsT=atT2[:, kt // (CH2 // P), h * (NK // 2) + (kt % (CH2 // P)), :],
                        rhs=vv_sb[:, kt, ds(h * D_HEAD, D_HEAD)],
                        start=(kt == 0 and h == 0),
                        stop=(kt == NK - 1 and h == 1),
                        skip_group_check=True,
                    )
            o3 = blk.tile([P, DH2], BF16, tag="o3", name="o3")
            for h in range(2):
                nc.vector.tensor_scalar_mul(
                    o3[:, ds(h * D_HEAD, D_HEAD)],
                    po[:, ds(h * D_HEAD, D_HEAD)],
                    recs[h][:, 0:1],
                )
            o2 = blk.tile([DH2, P], BF16, tag="o2", bufs=2, name="o2")
            ring(xbar_ring).dma_start_transpose(o2[:], o3[:])
            return o2

        def out_proj(ib, o2):
            pw = ps.tile([P, D_MODEL], FP32, tag="pw", name="pw")
            nc.tensor.matmul(pw[:], lhsT=o2[:], rhs=Wo_sb[:], start=True, stop=True)
            fin = blk.tile([P, D_MODEL], BF16, tag="fin", name="fin")
            nc.vector.tensor_copy(fin[:], pw[:])
            ring(out_ring).dma_start(out_partial[ts(ib, P), :], fin[:])

        # ---- software-pipelined main loop (pos 3 ahead, sh 1 ahead) ----
        for pib in (0, 1, 2):
            fr = new_frags()
            pos_chunks(pib, range(NCH), fr)
            pb_write(pib, fr)
        sh_cur = sh_read(0)
        # pipe = (ready_for_attnv, just_computed, ready_for_proj)
        pipe = (None, None, None)
        for ib in range(NB):
            sh_next = sh_read(ib + 1) if ib + 1 < NB else None
            fr = new_frags() if ib + 3 < NB else None
            sums2 = []
            at2 = blk.tile([P, 2, 2, CH2], BF16, tag="at2", bufs=3, name="at2")
            atT2 = blk.tile([P, 2, NK, P], BF16, tag="atT2", bufs=3, name="atT2")
            for h in range(2):
                sums2.append(blk.tile([P, 2], FP32, tag=f"sums{h}", bufs=3, name=f"sums{h}"))
            for half in range(S // CH2):
                pcs = [
                    ps.tile([P, CH2], FP32, tag="pc", bufs=2, name=f"pc{h}")
                    for h in range(2)
                ]
                for seg in range(CH2 // CH):
                    cols = ds(half * CH2 + seg * CH, CH)
                    for h in range(2):
                        nc.tensor.matmul(
                            pcs[h][:, ts(seg, CH)],
                            lhsT=qTu[ds(h * D_HEAD, D_HEAD), ts(ib, P)],
                            rhs=kT[ds(h * D_HEAD, D_HEAD), cols],
                            start=True,
                            stop=False,
                        )
                    for h in range(2):
                        nc.tensor.matmul(
                            pcs[h][:, ts(seg, CH)],
                            lhsT=identsh[:],
                            rhs=sh_cur[h][:, cols],
                            start=False,
                            stop=True,
                            skip_group_check=True,
                        )
                if fr is not None:
                    pos_chunks(ib + 3, (2 * half, 2 * half + 1), fr)
                for h in range(2):
                    nc.scalar.activation(
                        at2[:, half, h, :],
                        pcs[h][:],
                        Exp,
                        accum_out=sums2[h][:, ds(half, 1)],
                    )
                ring(xbar_ring).dma_start_transpose(
                    atT2[:, half, :, :], at2[:, half, :, :]
                )
            recs = []
            for h in range(2):
                stot = blk.tile([P, 1], FP32, tag=f"stot{h}", name=f"stot{h}")
                nc.vector.tensor_tensor(stot[:], sums2[h][:, 0:1], sums2[h][:, 1:2], ADD)
                rec = blk.tile([P, 1], FP32, tag=f"rec{h}", bufs=3, name=f"rec{h}")
                nc.vector.reciprocal(rec[:], stot[:])
                recs.append(rec)
            if fr is not None:
                pb_write(ib + 3, fr)
            if sh_next is not None:
                sh_cur = sh_next
            if pipe[0] is not None:
                o2n = out_attnv(ib - 2, *pipe[0])
            else:
                o2n = None
            if pipe[2] is not None:
                out_proj(ib - 3, pipe[2])
            pipe = ((pipe[1], (atT2, recs)), o2n, pipe[1])[0] + (o2n,)
        # epilogue: finish blocks NB-2, NB-1 attnv and NB-3.. proj
        o2a = out_attnv(NB - 2, *pipe[0])
        out_proj(NB - 3, pipe[2])
        o2b = out_attnv(NB - 1, *pipe[1])
        out_proj(NB - 2, o2a)
        out_proj(NB - 1, o2b)

    nc.finalize()
    return nc


# ---------------- host side ----------------

USE_V2 = True

_NC_CACHE = {}


def _get_nc(S=S_FULL):
    key = (S, USE_V2)
    if key not in _NC_CACHE:
        _NC_CACHE[key] = build_nc2(S) if USE_V2 else build_nc(S)
    return _NC_CACHE[key]


def make_in_maps(inputs, S=S_FULL, n_cores=8):
    x = np.ascontiguousarray(np.asarray(inputs["x"], np.float32))
    pos = np.ascontiguousarray(np.asarray(inputs["pos_embedding"], np.float32))
    Wq = np.asarray(inputs["Wq"], np.float32)
    bq = np.asarray(inputs["bq"], np.float32)
    Wk = np.asarray(inputs["Wk"], np.float32)
    Wv = np.asarray(inputs["Wv"], np.float32)
    Wp = np.asarray(inputs["Wp"], np.float32)
    u = np.asarray(inputs["u"], np.float32)
    v = np.asarray(inputs["v"], np.float32)
    Wo = np.asarray(inputs["Wo"], np.float32)

    in_maps = []
    for c in range(n_cores):
        b = c // 4
        h0 = 2 * (c % 4)
        sl = slice(h0 * D_HEAD, (h0 + 2) * D_HEAD)
        u_eff = ((u[h0 : h0 + 2].reshape(-1) + bq[sl]) * ISQ).astype(np.float32)
        v_eff = ((v[h0 : h0 + 2].reshape(-1) + bq[sl]) * ISQ).astype(np.float32)
        m = {
            "xT": np.ascontiguousarray(x[b, :S].T).astype(BF16NP),
            "posT": np.ascontiguousarray(pos[b, :S].T).astype(BF16NP),
            "Wq": np.ascontiguousarray(Wq[:, sl]).astype(BF16NP),
            "Wk": np.ascontiguousarray(Wk[:, sl]).astype(BF16NP),
            "Wv": np.ascontiguousarray(Wv[:, sl]).astype(BF16NP),
            "Wp": np.ascontiguousarray(Wp[:, sl]).astype(BF16NP),
            "ueff": u_eff.reshape(DH2, 1),
            "veff": v_eff.reshape(DH2, 1),
        }
        if USE_V2:
            m["Wo128"] = np.ascontiguousarray(Wo[sl, :]).astype(BF16NP)
        else:
            m["Wo2"] = np.ascontiguousarray(
                Wo[sl, :].reshape(2, D_HEAD, D_MODEL)
            ).astype(BF16NP)
        in_maps.append(m)
    return in_maps


def assemble(inputs, results, S=S_FULL):
    bv = np.asarray(inputs["bv"], np.float64)
    Wo = np.asarray(inputs["Wo"], np.float64)
    bo = np.asarray(inputs["bo"], np.float64)
    const = (bv @ Wo + bo).astype(np.float32)
    out = np.zeros((B_FULL, S, D_MODEL), np.float32)
    for c, res in enumerate(results):
        out[c // 4] += np.asarray(res["out_partial"], np.float32)
    out += const[None, None, :]
    return out


def _run(inputs, trace=False, **kw):
    nc = _get_nc(S_FULL)
    in_maps = make_in_maps(inputs, S_FULL)
    res = run_bass_kernel_spmd(nc, in_maps, list(range(8)), trace=trace, **kw)
    out = assemble(inputs, res.results, S_FULL)
    return out, res


def kernel(**inputs) -> np.ndarray:
    out, _ = _run(inputs, trace=False)
    return out



# revision 18
# speedup vs baseline: 1.1952x; 1.1952x over previous
"""Trainium2 Bass kernel for Transformer-XL style relative-position MHSA.

Problem: nn_MultiHeadSelfAttention_14989435863450
  B=2, S=2048, D=512, H=8, dh=64, fp32 I/O.

Sharding (8 cores): core c -> batch b = c//4, head pair h0 = 2*(c%4).
Each core computes its 2 heads' attention and the partial output
projection (out_slice @ Wo[slice]); host sums 4 partials per batch and
adds the constant (bv @ Wo + bo) row vector.

Math folds (exact):
  - bq folds into u,v:  u_eff = (u + bq) / sqrt(D)
  - bk adds a per-query-row constant to scores -> cancels in softmax
  - bv contributes attn-weighted 1 * bv = bv -> host-side constant
  - 1/sqrt(D) folded into q at evacuation time

Relative shift: pos scores are streamed to a DRAM buffer PB[S, S+1]
with rows [0 | posrow_i]; reading PB.flat[S : S + S*S] as [S, S] is
exactly Transformer-XL's pad-reshape-slice shift (including the wrap).
"""

import math
from contextlib import ExitStack

import ml_dtypes
import numpy as np

BF16NP = ml_dtypes.bfloat16

import concourse.bass as bass
import concourse.bacc as bacc_mod
import concourse.mybir as mybir
import concourse.tile as tile
from concourse.bass import ts, ds
from concourse.bass_utils import run_bass_kernel_spmd
from concourse.masks import make_identity

FP32 = mybir.dt.float32
F32R = mybir.dt.float32r
BF16 = mybir.dt.bfloat16

D_MODEL = 512
NUM_HEADS = 8
D_HEAD = 64
DH2 = 2 * D_HEAD  # head-pair width per core
B_FULL = 2
S_FULL = 2048
P = 128
ISQ = 1.0 / math.sqrt(D_MODEL)

Exp = mybir.ActivationFunctionType.Exp
ADD = mybir.AluOpType.add
MULT = mybir.AluOpType.mult


def build_nc(S=S_FULL, use_f32r=False, reps=1, accum_shift=False, transpose_ring='sp', shift_ring='sp', pb_ring='sp', out_ring='sp', fake_transpose=False, pe_transpose=True):
    """Build the single-core Bass program (SPMD: same program, 8 cores)."""
    nc = bacc_mod.Bacc()
    NB = S // P          # query blocks
    NK = S // P          # key tiles
    CH = min(512, S)     # score column chunk (PSUM bank)
    NCH = S // CH        # chunks per row
    KD = D_MODEL // P    # contraction tiles over D

    MMDT = F32R if use_f32r else BF16

    xT = nc.declare_dram_parameter("xT", [D_MODEL, S], MMDT, isOutput=False)
    posT = nc.declare_dram_parameter("posT", [D_MODEL, S], MMDT, isOutput=False)
    Wq = nc.declare_dram_parameter("Wq", [D_MODEL, DH2], MMDT, isOutput=False)
    Wk = nc.declare_dram_parameter("Wk", [D_MODEL, DH2], MMDT, isOutput=False)
    Wv = nc.declare_dram_parameter("Wv", [D_MODEL, DH2], MMDT, isOutput=False)
    Wp = nc.declare_dram_parameter("Wp", [D_MODEL, DH2], MMDT, isOutput=False)
    Wo2 = nc.declare_dram_parameter("Wo2", [2, D_HEAD, D_MODEL], MMDT, isOutput=False)
    ueff = nc.declare_dram_parameter("ueff", [DH2, 1], FP32, isOutput=False)
    veff = nc.declare_dram_parameter("veff", [DH2, 1], FP32, isOutput=False)
    out_partial = nc.declare_dram_parameter("out_partial", [S, D_MODEL], FP32, isOutput=True)

    with ExitStack() as ctx:
        tc = ctx.enter_context(tile.TileContext(nc))
        consts = ctx.enter_context(tc.tile_pool(name="consts", bufs=1))
        blk = ctx.enter_context(tc.tile_pool(name="blk", bufs=3))
        dram = ctx.enter_context(tc.tile_pool(name="dram", bufs=1, space="DRAM"))
        psA = ctx.enter_context(tc.tile_pool(name="psA", bufs=1, space="PSUM"))
        psB = ctx.enter_context(tc.tile_pool(name="psB", bufs=1 if pe_transpose else 2, space="PSUM"))
        psE = ctx.enter_context(tc.tile_pool(name="psE", bufs=1, space="PSUM")) if pe_transpose else None
        psC = ctx.enter_context(tc.tile_pool(name="psC", bufs=1, space="PSUM"))
        psD = ctx.enter_context(tc.tile_pool(name="psD", bufs=1, space="PSUM"))

        # ---- load constants / inputs ----
        xT_sb = consts.tile([P, KD, S], MMDT)
        nc.sync.dma_start(xT_sb[:], xT.rearrange("(o p) s -> p o s", p=P))
        posT_sb = consts.tile([P, KD, S], MMDT)
        nc.sync.dma_start(posT_sb[:], posT.rearrange("(o p) s -> p o s", p=P))
        w_sbs = {}
        for nm, handle in (("Wq", Wq), ("Wk", Wk), ("Wv", Wv), ("Wp", Wp)):
            w_sb = consts.tile([P, KD, DH2], MMDT, name=f"{nm}_sb")
            nc.sync.dma_start(w_sb[:], handle.rearrange("(o p) m -> p o m", p=P))
            w_sbs[nm] = w_sb
        Wo_sb = consts.tile([D_HEAD, 2, D_MODEL], MMDT)
        nc.sync.dma_start(Wo_sb[:], Wo2.rearrange("t p n -> p t n"))
        ident_sb = consts.tile([P, P], BF16)
        make_identity(nc, ident_sb[:])
        ueff_sb = consts.tile([DH2, 1], FP32)
        nc.sync.dma_start(ueff_sb[:], ueff[:, :])
        veff_sb = consts.tile([DH2, 1], FP32)
        nc.sync.dma_start(veff_sb[:], veff[:, :])

        # ---- projections ----
        # qT/kT/pT: [DH2, S] = W.T @ x  (lhsT = W [D, DH2], rhs = xT [D, S])
        def proj_T(w_sb, src_sb):
            pq = psA.tile([P, S], FP32, tag="psA", name="pq")
            for chn in range(NCH):
                for kt in range(KD):
                    nc.tensor.matmul(
                        pq[:, ts(chn, CH)],
                        lhsT=w_sb[:, kt, :],
                        rhs=src_sb[:, kt, ts(chn, CH)],
                        start=(kt == 0),
                        stop=(kt == KD - 1),
                    )
            return pq

        qTu = consts.tile([DH2, S], MMDT)
        qTv = consts.tile([DH2, S], MMDT)
        pq = proj_T(w_sbs["Wq"], xT_sb)
        nc.vector.tensor_scalar(qTu[:], pq[:], ISQ, ueff_sb[:, 0:1], MULT, ADD)
        nc.vector.tensor_scalar(qTv[:], pq[:], ISQ, veff_sb[:, 0:1], MULT, ADD)
        kT = consts.tile([DH2, S], MMDT)
        pk = proj_T(w_sbs["Wk"], xT_sb)
        nc.scalar.copy(kT[:], pk[:])
        pT = consts.tile([DH2, S], MMDT)
        pp_ = proj_T(w_sbs["Wp"], posT_sb)
        nc.scalar.copy(pT[:], pp_[:])

        # vv: [S, DH2] natural (bf16 for the attn@v matmuls)
        vv_sb = consts.tile([P, NK, DH2], BF16)
        for st in range(NK):
            pv = psB.tile([P, CH], FP32, tag="psB", name="pv")
            for kt in range(KD):
                nc.tensor.matmul(
                    pv[:, :DH2],
                    lhsT=xT_sb[:, kt, ts(st, P)],
                    rhs=w_sbs["Wv"][:, kt, :],
                    start=(kt == 0),
                    stop=(kt == KD - 1),
                )
            nc.any.tensor_copy(vv_sb[:, st, :], pv[:, :DH2])

        # ---- per-head DRAM pos-score buffers (padded for the rel-shift) ----
        PB = [
            dram.tile([S, S + 1], BF16, name=f"pb{h}")
            for h in range(2)
        ]

        def pos_block(h, ib):
            """pos scores for q rows [128*ib, 128*ib+128) -> PB[h] rows."""
            pe = blk.tile([P, S + 1], BF16, tag="posext", name="pe")
            nc.vector.memset(pe[:, 0:1], 0.0)
            for chn in range(NCH):
                pp = psB.tile([P, CH], FP32, tag="psB", name="pp")
                nc.tensor.matmul(
                    pp[:],
                    lhsT=qTv[ds(h * D_HEAD, D_HEAD), ts(ib, P)],
                    rhs=pT[ds(h * D_HEAD, D_HEAD), ts(chn, CH)],
                    start=True,
                    stop=True,
                )
                if chn % 2 == 0:
                    nc.scalar.copy(pe[:, 1 + chn * CH : 1 + (chn + 1) * CH], pp[:])
                else:
                    nc.vector.tensor_copy(pe[:, 1 + chn * CH : 1 + (chn + 1) * CH], pp[:])
            (nc.scalar if pb_ring == 'act' else nc.sync).dma_start(PB[h][ts(ib, P), :], pe[:])

        def head_block(h, ib):
            """content + shifted pos + softmax + transpose for one q block."""
            # content matmuls -> PSUM -> evac (bf16), then the shifted pos
            # read is an SWDGE accumulate-DMA straight onto the content tile.
            pc = psA.tile([P, S], FP32, tag="psA", name="pc")
            for chn in range(NCH):
                nc.tensor.matmul(
                    pc[:, ts(chn, CH)],
                    lhsT=qTu[ds(h * D_HEAD, D_HEAD), ts(ib, P)],
                    rhs=kT[ds(h * D_HEAD, D_HEAD), ts(chn, CH)],
                    start=True,
                    stop=True,
                )
            sc = blk.tile([P, S], BF16, tag="scores", name="sc")
            flat = PB[h].flatten()
            view = flat[ds(S + ib * P * S, P * S)].rearrange("(p s) -> p s", s=S)
            if accum_shift:
                nc.vector.tensor_copy(sc[:], pc[:])
                nc.gpsimd.dma_start(sc[:], view, accum_op=ADD)
            else:
                sh = blk.tile([P, S], BF16, tag="shift", name="sh")
                if shift_ring == "pool":
                    nc.gpsimd.dma_start(sh[:], view)
                else:
                    nc.sync.dma_start(sh[:], view)
                nc.vector.tensor_tensor(sc[:], pc[:], sh[:], ADD)
            at = blk.tile([P, S], BF16, tag="attn", name="at")
            sums = blk.tile([P, 1], FP32, tag="sums", name="sums")
            nc.scalar.activation(at[:], sc[:], Exp, accum_out=sums[:])
            rec = blk.tile([P, 1], FP32, tag="rec", name="rec")
            nc.vector.reciprocal(rec[:], sums[:])
            nc.vector.tensor_scalar_mul(at[:], at[:], rec[:, 0:1])
            atT = blk.tile([P, NK, P], BF16, tag=f"attnT{h}", name="atT")
            if pe_transpose:
                G = min(4, NK)
                for g in range(NK // G):
                    pt4 = psE.tile([P, G, P], BF16, tag="psE", name="pt4")
                    for j in range(G):
                        kt = G * g + j
                        nc.tensor.transpose(pt4[:, j, :], at[:, ts(kt, P)], ident_sb[:])
                    if g % 2 == 0:
                        nc.vector.tensor_copy(atT[:, ts(g, G), :], pt4[:])
                    else:
                        nc.scalar.copy(atT[:, ts(g, G), :], pt4[:])
            elif fake_transpose:
                nc.sync.dma_start(atT[:], at[:].rearrange("p (a b) -> p a b", b=P))
            else:
                (nc.scalar if transpose_ring == 'act' else nc.sync).dma_start_transpose(atT[:], at[:])
            return atT

        def out_block(ib, atTs):
            o2s = []
            for h in range(2):
                po = psC.tile([D_HEAD, P], FP32, tag="psC", name="po")
                for kt in range(NK):
                    nc.tensor.matmul(
                        po[:],
                        lhsT=vv_sb[:, kt, ds(h * D_HEAD, D_HEAD)],
                        rhs=atTs[h][:, kt, :],
                        start=(kt == 0),
                        stop=(kt == NK - 1),
                    )
                o2 = blk.tile([D_HEAD, P], MMDT, tag=f"o2_{h}", name="o2")
                nc.scalar.copy(o2[:], po[:])
                o2s.append(o2)
            pw = psD.tile([P, D_MODEL], FP32, tag="psD", name="pw")
            for h in range(2):
                nc.tensor.matmul(
                    pw[:],
                    lhsT=o2s[h][:],
                    rhs=Wo_sb[:, h, :],
                    start=(h == 0),
                    stop=(h == 1),
                )
            fin = blk.tile([P, D_MODEL], FP32, tag="fin", name="fin")
            nc.vector.tensor_copy(fin[:], pw[:])
            (nc.scalar if out_ring == 'act' else nc.sync).dma_start(out_partial[ts(ib, P), :], fin[:])

        # ---- main pipeline ----
        def pipeline():
            for h in range(2):
                pos_block(h, 0)
            for ib in range(NB):
                atTs = []
                for h in range(2):
                    if ib + 1 < NB:
                        pos_block(h, ib + 1)
                    atTs.append(head_block(h, ib))
                out_block(ib, atTs)

        if reps == 1:
            pipeline()
        else:
            with tc.For_i(0, reps, 1):
                pipeline()

    nc.finalize()
    return nc


FP8 = mybir.dt.float8e4


def build_nc2(
    S=S_FULL,
    pb_dt=FP8,
    posev="vvvvvvvv",  # evac engine per (chn,h) slot: v=vector, s=scalar
    sh_ring="sp",
    pb_ring="act",
    xbar_ring="sp",
    out_ring="act",
):
    """v2: exp-from-PSUM via identity-accum, row-tiled scores, [q,dh] attn@v
    with norm folded into the per-partition o3 evac, merged out-proj,
    software-pipelined pos(+2)/sh(+1)/content(ib)/output(ib-1)."""
    nc = bacc_mod.Bacc()
    NB = S // P
    NK = S // P
    CH = 512
    NCH = S // CH
    CH2 = 1024
    KD = D_MODEL // P

    xT = nc.declare_dram_parameter("xT", [D_MODEL, S], BF16, isOutput=False)
    posT = nc.declare_dram_parameter("posT", [D_MODEL, S], BF16, isOutput=False)
    Wq = nc.declare_dram_parameter("Wq", [D_MODEL, DH2], BF16, isOutput=False)
    Wk = nc.declare_dram_parameter("Wk", [D_MODEL, DH2], BF16, isOutput=False)
    Wv = nc.declare_dram_parameter("Wv", [D_MODEL, DH2], BF16, isOutput=False)
    Wp = nc.declare_dram_parameter("Wp", [D_MODEL, DH2], BF16, isOutput=False)
    Wo128 = nc.declare_dram_parameter("Wo128", [DH2, D_MODEL], BF16, isOutput=False)
    ueff = nc.declare_dram_parameter("ueff", [DH2, 1], FP32, isOutput=False)
    veff = nc.declare_dram_parameter("veff", [DH2, 1], FP32, isOutput=False)
    out_partial = nc.declare_dram_parameter("out_partial", [S, D_MODEL], BF16, isOutput=True)

    def ring(which):
        return nc.scalar if which == "act" else nc.sync

    with ExitStack() as ctx:
        tc = ctx.enter_context(tile.TileContext(nc))
        consts = ctx.enter_context(tc.tile_pool(name="consts", bufs=1))
        blk = ctx.enter_context(tc.tile_pool(name="blk", bufs=2))
        dram = ctx.enter_context(tc.tile_pool(name="dram", bufs=1, space="DRAM"))
        ps = ctx.enter_context(tc.tile_pool(name="ps", bufs=1, space="PSUM"))

        # ---- load constants / inputs ----
        xT_sb = consts.tile([P, KD, S], BF16)
        nc.sync.dma_start(xT_sb[:], xT.rearrange("(o p) s -> p o s", p=P))
        posT_sb = consts.tile([P, KD, S], BF16)
        nc.sync.dma_start(posT_sb[:], posT.rearrange("(o p) s -> p o s", p=P))
        w_sbs = {}
        for nm, handle in (("Wq", Wq), ("Wk", Wk), ("Wv", Wv), ("Wp", Wp)):
            w_sb = consts.tile([P, KD, DH2], BF16, name=f"{nm}_sb")
            nc.sync.dma_start(w_sb[:], handle.rearrange("(o p) m -> p o m", p=P))
            w_sbs[nm] = w_sb
        Wo_sb = consts.tile([DH2, D_MODEL], BF16)
        nc.sync.dma_start(Wo_sb[:], Wo128[:, :])
        identsh = consts.tile([P, P], pb_dt)
        make_identity(nc, identsh[:])
        identb = consts.tile([P, P], BF16)
        make_identity(nc, identb[:])
        ueff_sb = consts.tile([DH2, 1], FP32)
        nc.sync.dma_start(ueff_sb[:], ueff[:, :])
        veff_sb = consts.tile([DH2, 1], FP32)
        nc.sync.dma_start(veff_sb[:], veff[:, :])

        qTu = consts.tile([DH2, S], BF16)
        qTv = consts.tile([DH2, S], BF16)
        kT = consts.tile([DH2, S], BF16)
        pT = consts.tile([DH2, S], BF16)

        # ---- projections (reuse steady-state PSUM tags) ----
        def proj_T(w_sb, src_sb, dsts):
            for half in range(S // CH2):
                pq = ps.tile([P, CH2], FP32, tag="pc", bufs=2, name="pq")
                for seg in range(CH2 // CH):
                    for kt in range(KD):
                        nc.tensor.matmul(
                            pq[:, ts(seg, CH)],
                            lhsT=w_sb[:, kt, :],
                            rhs=src_sb[:, kt, ds(half * CH2 + seg * CH, CH)],
                            start=(kt == 0),
                            stop=(kt == KD - 1),
                        )
                for dst, scale, bias in dsts:
                    if scale is None:
                        nc.scalar.copy(dst[:, ts(half, CH2)], pq[:])
                    else:
                        nc.vector.tensor_scalar(
                            dst[:, ts(half, CH2)], pq[:], scale, bias, MULT, ADD
                        )

        proj_T(w_sbs["Wq"], xT_sb, [(qTu, ISQ, ueff_sb[:, 0:1]), (qTv, ISQ, veff_sb[:, 0:1])])
        proj_T(w_sbs["Wk"], xT_sb, [(kT, None, None)])
        proj_T(w_sbs["Wp"], posT_sb, [(pT, None, None)])

        vv_sb = consts.tile([P, NK, DH2], BF16)
        for st in range(NK):
            pv = ps.tile([P, DH2], FP32, tag=("po" if st % 2 == 0 else "pw"), bufs=1, name="pv")
            for kt in range(KD):
                nc.tensor.matmul(
                    pv[:],
                    lhsT=xT_sb[:, kt, ts(st, P)],
                    rhs=w_sbs["Wv"][:, kt, :],
                    start=(kt == 0),
                    stop=(kt == KD - 1),
                )
            nc.any.tensor_copy(vv_sb[:, st, :], pv[:])

        # ---- per-head DRAM pos-score buffers, rows [0 | posrow] ----
        PB = [dram.tile([S, S + 1], pb_dt, name=f"pb{h}") for h in range(2)]

        def new_frags():
            fr = []
            for h in range(2):
                f = blk.tile([P, S + 1], pb_dt, tag=f"frag{h}", name=f"frag{h}")
                nc.gpsimd.memset(f[:, 0:1], 0.0)
                fr.append(f)
            return fr

        def pos_chunks(ib, chns, frags):
            for chn in chns:
                for h in range(2):
                    pp = ps.tile([P, CH], FP32, tag="pp", bufs=2, name="pp")
                    nc.tensor.matmul(
                        pp[:],
                        lhsT=qTv[ds(h * D_HEAD, D_HEAD), ts(ib, P)],
                        rhs=pT[ds(h * D_HEAD, D_HEAD), ts(chn, CH)],
                        start=True,
                        stop=True,
                    )
                    dst = frags[h][:, 1 + chn * CH : 1 + (chn + 1) * CH]
                    if posev[chn * 2 + h] == "v":
                        nc.vector.tensor_copy(dst, pp[:])
                    else:
                        nc.scalar.copy(dst, pp[:])

        def pb_write(ib, frags):
            for h in range(2):
                ring(pb_ring).dma_start(PB[h][ts(ib, P), :], frags[h][:])

        def sh_read(ib):
            out = []
            for h in range(2):
                shh = blk.tile([P, S], pb_dt, tag=f"sh{h}", name=f"sh{h}")
                flat = PB[h].flatten()
                view = flat[ds(S + ib * P * S, P * S)].rearrange("(p s) -> p s", s=S)
                for half in range(2):
                    cols = ds(half * (S // 2), S // 2)
                    ring(sh_ring).dma_start(shh[:, cols], view[:, cols])
                out.append(shh)
            return out

        def out_attnv(ib, atT2, recs):
            # single PSUM bank, ONE start group: only the very first MM has
            # start=True (a start resets the whole bank)
            po = ps.tile([P, DH2], FP32, tag="po", name="po")
            for kt in range(NK):
                for h in range(2):
                    nc.tensor.matmul(
                        po[:, ds(h * D_HEAD, D_HEAD)],
                        lhsT=atT2[:, kt // (CH2 // P), h * (NK // 2) + (kt % (CH2 // P)), :],
                        rhs=vv_sb[:, kt, ds(h * D_HEAD, D_HEAD)],
                        start=(kt == 0 and h == 0),
                        stop=(kt == NK - 1 and h == 1),
                        skip_group_check=True,
                    )
            o3 = blk.tile([P, DH2], BF16, tag="o3", name="o3")
            for h in range(2):
                nc.vector.tensor_scalar_mul(
                    o3[:, ds(h * D_HEAD, D_HEAD)],
                    po[:, ds(h * D_HEAD, D_HEAD)],
                    recs[h][:, 0:1],
                )
            o2 = blk.tile([DH2, P], BF16, tag="o2", bufs=2, name="o2")
            ring(xbar_ring).dma_start_transpose(o2[:], o3[:])
            return o2

        def out_proj(ib, o2):
            pw = ps.tile([P, D_MODEL], FP32, tag="pw", name="pw")
            nc.tensor.matmul(pw[:], lhsT=o2[:], rhs=Wo_sb[:], start=True, stop=True)
            fin = blk.tile([P, D_MODEL], BF16, tag="fin", name="fin")
            nc.vector.tensor_copy(fin[:], pw[:])
            ring(out_ring).dma_start(out_partial[ts(ib, P), :], fin[:])

        # ---- software-pipelined main loop (pos 3 ahead, sh 1 ahead) ----
        for pib in (0, 1, 2):
            fr = new_frags()
            pos_chunks(pib, range(NCH), fr)
            pb_write(pib, fr)
        sh_cur = sh_read(0)
        # pipe = (ready_for_attnv, just_computed, ready_for_proj)
        pipe = (None, None, None)
        for ib in range(NB):
            sh_next = sh_read(ib + 1) if ib + 1 < NB else None
            fr = new_frags() if ib + 3 < NB else None
            sums2 = []
            at2 = blk.tile([P, 2, 2, CH2], BF16, tag="at2", bufs=3, name="at2")
            atT2 = blk.tile([P, 2, NK, P], BF16, tag="atT2", bufs=3, name="atT2")
            for h in range(2):
                sums2.append(blk.tile([P, 2], FP32, tag=f"sums{h}", bufs=3, name=f"sums{h}"))
            for half in range(S // CH2):
                pcs = [
                    ps.tile([P, CH2], FP32, tag="pc", bufs=2, name=f"pc{h}")
                    for h in range(2)
                ]
                for seg in range(CH2 // CH):
                    cols = ds(half * CH2 + seg * CH, CH)
                    for h in range(2):
                        nc.tensor.matmul(
                            pcs[h][:, ts(seg, CH)],
                            lhsT=qTu[ds(h * D_HEAD, D_HEAD), ts(ib, P)],
                            rhs=kT[ds(h * D_HEAD, D_HEAD), cols],
                            start=True,
                            stop=False,
                        )
                    for h in range(2):
                        nc.tensor.matmul(
                            pcs[h][:, ts(seg, CH)],
                            lhsT=identsh[:],
                            rhs=sh_cur[h][:, cols],
                            start=False,
                            stop=True,
                            skip_group_check=True,
                        )
                if fr is not None:
                    pos_chunks(ib + 3, (2 * half, 2 * half + 1), fr)
                for h in range(2):
                    nc.scalar.activation(
                        at2[:, half, h, :],
                        pcs[h][:],
                        Exp,
                        accum_out=sums2[h][:, ds(half, 1)],
                    )
                ring(xbar_ring).dma_start_transpose(
                    atT2[:, half, :, :], at2[:, half, :, :]
                )
            recs = []
            for h in range(2):
                stot = blk.tile([P, 1], FP32, tag=f"stot{h}", name=f"stot{h}")
                nc.vector.tensor_tensor(stot[:], sums2[h][:, 0:1], sums2[h][:, 1:2], ADD)
                rec = blk.tile([P, 1], FP32, tag=f"rec{h}", bufs=3, name=f"rec{h}")
                nc.vector.reciprocal(rec[:], stot[:])
                recs.append(rec)
            if fr is not None:
                pb_write(ib + 3, fr)
            if sh_next is not None:
                sh_cur = sh_next
            if pipe[0] is not None:
                o2n = out_attnv(ib - 2, *pipe[0])
            else:
                o2n = None
            if pipe[2] is not None:
                out_proj(ib - 3, pipe[2])
            pipe = ((pipe[1], (atT2, recs)), o2n, pipe[1])[0] + (o2n,)
        # epilogue: finish blocks NB-2, NB-1 attnv and NB-3.. proj
        o2a = out_attnv(NB - 2, *pipe[0])
        out_proj(NB - 3, pipe[2])
        o2b = out_attnv(NB - 1, *pipe[1])
        out_proj(NB - 2, o2a)
        out_proj(NB - 1, o2b)

    nc.finalize()
    return nc


# ---------------- host side ----------------

USE_V2 = True

_NC_CACHE = {}


def _get_nc(S=S_FULL):
    key = (S, USE_V2)
    if key not in _NC_CACHE:
        _NC_CACHE[key] = build_nc2(S) if USE_V2 else build_nc(S)
    return _NC_CACHE[key]


def make_in_maps(inputs, S=S_FULL, n_cores=8):
    x = np.ascontiguousarray(np.asarray(inputs["x"], np.float32))
    pos = np.ascontiguousarray(np.asarray(inputs["pos_embedding"], np.float32))
    Wq = np.asarray(inputs["Wq"], np.float32)
    bq = np.asarray(inputs["bq"], np.float32)
    Wk = np.asarray(inputs["Wk"], np.float32)
    Wv = np.asarray(inputs["Wv"], np.float32)
    Wp = np.asarray(inputs["Wp"], np.float32)
    u = np.asarray(inputs["u"], np.float32)
    v = np.asarray(inputs["v"], np.float32)
    Wo = np.asarray(inputs["Wo"], np.float32)

    in_maps = []
    for c in range(n_cores):
        b = c // 4
        h0 = 2 * (c % 4)
        sl = slice(h0 * D_HEAD, (h0 + 2) * D_HEAD)
        u_eff = ((u[h0 : h0 + 2].reshape(-1) + bq[sl]) * ISQ).astype(np.float32)
        v_eff = ((v[h0 : h0 + 2].reshape(-1) + bq[sl]) * ISQ).astype(np.float32)
        m = {
            "xT": np.ascontiguousarray(x[b, :S].T).astype(BF16NP),
            "posT": np.ascontiguousarray(pos[b, :S].T).astype(BF16NP),
            "Wq": np.ascontiguousarray(Wq[:, sl]).astype(BF16NP),
            "Wk": np.ascontiguousarray(Wk[:, sl]).astype(BF16NP),
            "Wv": np.ascontiguousarray(Wv[:, sl]).astype(BF16NP),
            "Wp": np.ascontiguousarray(Wp[:, sl]).astype(BF16NP),
            "ueff": u_eff.reshape(DH2, 1),
            "veff": v_eff.reshape(DH2, 1),
        }
        if USE_V2:
            m["Wo128"] = np.ascontiguousarray(Wo[sl, :]).astype(BF16NP)
        else:
            m["Wo2"] = np.ascontiguousarray(
                Wo[sl, :].reshape(2, D_HEAD, D_MODEL)
            ).astype(BF16NP)
        in_maps.append(m)
    return in_maps


def assemble(inputs, results, S=S_FULL):
    bv = np.asarray(inputs["bv"], np.float64)
    Wo = np.asarray(inputs["Wo"], np.float64)
    bo = np.asarray(inputs["bo"], np.float64)
    const = (bv @ Wo + bo).astype(np.float32)
    out = np.zeros((B_FULL, S, D_MODEL), np.float32)
    for c, res in enumerate(results):
        out[c // 4] += np.asarray(res["out_partial"], np.float32)
    out += const[None, None, :]
    return out


def _run(inputs, trace=False, **kw):
    nc = _get_nc(S_FULL)
    in_maps = make_in_maps(inputs, S_FULL)
    res = run_bass_kernel_spmd(nc, in_maps, list(range(8)), trace=trace, **kw)
    out = assemble(inputs, res.results, S_FULL)
    return out, res


def kernel(**inputs) -> np.ndarray:
    out, _ = _run(inputs, trace=False)
    return out



# revision 27
# speedup vs baseline: 1.4361x; 1.2015x over previous
"""Trainium2 Bass kernel for Transformer-XL style relative-position MHSA.

Problem: nn_MultiHeadSelfAttention_14989435863450
  B=2, S=2048, D=512, H=8, dh=64, fp32 I/O.

Sharding (8 cores): core c -> batch b = c//4, head pair h0 = 2*(c%4).
Each core computes its 2 heads' attention and the partial output
projection (out_slice @ Wo[slice]); host sums 4 partials per batch and
adds the constant (bv @ Wo + bo) row vector.

Math folds (exact):
  - bq folds into u,v:  u_eff = (u + bq) / sqrt(D)
  - bk adds a per-query-row constant to scores -> cancels in softmax
  - bv contributes attn-weighted 1 * bv = bv -> host-side constant
  - 1/sqrt(D) folded into q at evacuation time

Relative shift: pos scores are streamed to a DRAM buffer PB[S, S+1]
with rows [0 | posrow_i]; reading PB.flat[S : S + S*S] as [S, S] is
exactly Transformer-XL's pad-reshape-slice shift (including the wrap).

v2 (build_nc2, default): all-bf16 matmuls; PB in fp8e4; shifted pos is
added onto the content scores IN PSUM via an identity-matmul
accumulate, so exp reads PSUM directly (no separate add pass); softmax
normalization folds into the per-partition o3 evacuation of the
[q, dh2]-oriented attn@v; attn transposed via the DMA XBAR; merged
single out-projection; bf16 output partials. PSUM bank discipline: a
matmul with start=True resets its WHOLE bank, so concurrent
accumulation groups each get a private bank (or share one start).
419us -> 320us on 8 trn2 cores, rel err 1.4e-3.
"""

import math
from contextlib import ExitStack

import ml_dtypes
import numpy as np

BF16NP = ml_dtypes.bfloat16

import concourse.bass as bass
import concourse.bacc as bacc_mod
import concourse.mybir as mybir
import concourse.tile as tile
from concourse.bass import ts, ds
from concourse.bass_utils import run_bass_kernel_spmd
from concourse.masks import make_identity

FP32 = mybir.dt.float32
F32R = mybir.dt.float32r
BF16 = mybir.dt.bfloat16

D_MODEL = 512
NUM_HEADS = 8
D_HEAD = 64
DH2 = 2 * D_HEAD  # head-pair width per core
B_FULL = 2
S_FULL = 2048
P = 128
ISQ = 1.0 / math.sqrt(D_MODEL)

Exp = mybir.ActivationFunctionType.Exp
ADD = mybir.AluOpType.add
MULT = mybir.AluOpType.mult


def build_nc(S=S_FULL, use_f32r=False, reps=1, accum_shift=False, transpose_ring='sp', shift_ring='sp', pb_ring='sp', out_ring='sp', fake_transpose=False, pe_transpose=True):
    """Build the single-core Bass program (SPMD: same program, 8 cores)."""
    nc = bacc_mod.Bacc()
    NB = S // P          # query blocks
    NK = S // P          # key tiles
    CH = min(512, S)     # score column chunk (PSUM bank)
    NCH = S // CH        # chunks per row
    KD = D_MODEL // P    # contraction tiles over D

    MMDT = F32R if use_f32r else BF16

    xT = nc.declare_dram_parameter("xT", [D_MODEL, S], MMDT, isOutput=False)
    posT = nc.declare_dram_parameter("posT", [D_MODEL, S], MMDT, isOutput=False)
    Wq = nc.declare_dram_parameter("Wq", [D_MODEL, DH2], MMDT, isOutput=False)
    Wk = nc.declare_dram_parameter("Wk", [D_MODEL, DH2], MMDT, isOutput=False)
    Wv = nc.declare_dram_parameter("Wv", [D_MODEL, DH2], MMDT, isOutput=False)
    Wp = nc.declare_dram_parameter("Wp", [D_MODEL, DH2], MMDT, isOutput=False)
    Wo2 = nc.declare_dram_parameter("Wo2", [2, D_HEAD, D_MODEL], MMDT, isOutput=False)
    ueff = nc.declare_dram_parameter("ueff", [DH2, 1], FP32, isOutput=False)
    veff = nc.declare_dram_parameter("veff", [DH2, 1], FP32, isOutput=False)
    out_partial = nc.declare_dram_parameter("out_partial", [S, D_MODEL], FP32, isOutput=True)

    with ExitStack() as ctx:
        tc = ctx.enter_context(tile.TileContext(nc))
        consts = ctx.enter_context(tc.tile_pool(name="consts", bufs=1))
        blk = ctx.enter_context(tc.tile_pool(name="blk", bufs=3))
        dram = ctx.enter_context(tc.tile_pool(name="dram", bufs=1, space="DRAM"))
        psA = ctx.enter_context(tc.tile_pool(name="psA", bufs=1, space="PSUM"))
        psB = ctx.enter_context(tc.tile_pool(name="psB", bufs=1 if pe_transpose else 2, space="PSUM"))
        psE = ctx.enter_context(tc.tile_pool(name="psE", bufs=1, space="PSUM")) if pe_transpose else None
        psC = ctx.enter_context(tc.tile_pool(name="psC", bufs=1, space="PSUM"))
        psD = ctx.enter_context(tc.tile_pool(name="psD", bufs=1, space="PSUM"))

        # ---- load constants / inputs ----
        xT_sb = consts.tile([P, KD, S], MMDT)
        nc.sync.dma_start(xT_sb[:], xT.rearrange("(o p) s -> p o s", p=P))
        posT_sb = consts.tile([P, KD, S], MMDT)
        nc.sync.dma_start(posT_sb[:], posT.rearrange("(o p) s -> p o s", p=P))
        w_sbs = {}
        for nm, handle in (("Wq", Wq), ("Wk", Wk), ("Wv", Wv), ("Wp", Wp)):
            w_sb = consts.tile([P, KD, DH2], MMDT, name=f"{nm}_sb")
            nc.sync.dma_start(w_sb[:], handle.rearrange("(o p) m -> p o m", p=P))
            w_sbs[nm] = w_sb
        Wo_sb = consts.tile([D_HEAD, 2, D_MODEL], MMDT)
        nc.sync.dma_start(Wo_sb[:], Wo2.rearrange("t p n -> p t n"))
        ident_sb = consts.tile([P, P], BF16)
        make_identity(nc, ident_sb[:])
        ueff_sb = consts.tile([DH2, 1], FP32)
        nc.sync.dma_start(ueff_sb[:], ueff[:, :])
        veff_sb = consts.tile([DH2, 1], FP32)
        nc.sync.dma_start(veff_sb[:], veff[:, :])

        # ---- projections ----
        # qT/kT/pT: [DH2, S] = W.T @ x  (lhsT = W [D, DH2], rhs = xT [D, S])
        def proj_T(w_sb, src_sb):
            pq = psA.tile([P, S], FP32, tag="psA", name="pq")
            for chn in range(NCH):
                for kt in range(KD):
                    nc.tensor.matmul(
                        pq[:, ts(chn, CH)],
                        lhsT=w_sb[:, kt, :],
                        rhs=src_sb[:, kt, ts(chn, CH)],
                        start=(kt == 0),
                        stop=(kt == KD - 1),
                    )
            return pq

        qTu = consts.tile([DH2, S], MMDT)
        qTv = consts.tile([DH2, S], MMDT)
        pq = proj_T(w_sbs["Wq"], xT_sb)
        nc.vector.tensor_scalar(qTu[:], pq[:], ISQ, ueff_sb[:, 0:1], MULT, ADD)
        nc.vector.tensor_scalar(qTv[:], pq[:], ISQ, veff_sb[:, 0:1], MULT, ADD)
        kT = consts.tile([DH2, S], MMDT)
        pk = proj_T(w_sbs["Wk"], xT_sb)
        nc.scalar.copy(kT[:], pk[:])
        pT = consts.tile([DH2, S], MMDT)
        pp_ = proj_T(w_sbs["Wp"], posT_sb)
        nc.scalar.copy(pT[:], pp_[:])

        # vv: [S, DH2] natural (bf16 for the attn@v matmuls)
        vv_sb = consts.tile([P, NK, DH2], BF16)
        for st in range(NK):
            pv = psB.tile([P, CH], FP32, tag="psB", name="pv")
            for kt in range(KD):
                nc.tensor.matmul(
                    pv[:, :DH2],
                    lhsT=xT_sb[:, kt, ts(st, P)],
                    rhs=w_sbs["Wv"][:, kt, :],
                    start=(kt == 0),
                    stop=(kt == KD - 1),
                )
            nc.any.tensor_copy(vv_sb[:, st, :], pv[:, :DH2])

        # ---- per-head DRAM pos-score buffers (padded for the rel-shift) ----
        PB = [
            dram.tile([S, S + 1], BF16, name=f"pb{h}")
            for h in range(2)
        ]

        def pos_block(h, ib):
            """pos scores for q rows [128*ib, 128*ib+128) -> PB[h] rows."""
            pe = blk.tile([P, S + 1], BF16, tag="posext", name="pe")
            nc.vector.memset(pe[:, 0:1], 0.0)
            for chn in range(NCH):
                pp = psB.tile([P, CH], FP32, tag="psB", name="pp")
                nc.tensor.matmul(
                    pp[:],
                    lhsT=qTv[ds(h * D_HEAD, D_HEAD), ts(ib, P)],
                    rhs=pT[ds(h * D_HEAD, D_HEAD), ts(chn, CH)],
                    start=True,
                    stop=True,
                )
                if chn % 2 == 0:
                    nc.scalar.copy(pe[:, 1 + chn * CH : 1 + (chn + 1) * CH], pp[:])
                else:
                    nc.vector.tensor_copy(pe[:, 1 + chn * CH : 1 + (chn + 1) * CH], pp[:])
            (nc.scalar if pb_ring == 'act' else nc.sync).dma_start(PB[h][ts(ib, P), :], pe[:])

        def head_block(h, ib):
            """content + shifted pos + softmax + transpose for one q block."""
            # content matmuls -> PSUM -> evac (bf16), then the shifted pos
            # read is an SWDGE accumulate-DMA straight onto the content tile.
            pc = psA.tile([P, S], FP32, tag="psA", name="pc")
            for chn in range(NCH):
                nc.tensor.matmul(
                    pc[:, ts(chn, CH)],
                    lhsT=qTu[ds(h * D_HEAD, D_HEAD), ts(ib, P)],
                    rhs=kT[ds(h * D_HEAD, D_HEAD), ts(chn, CH)],
                    start=True,
                    stop=True,
                )
            sc = blk.tile([P, S], BF16, tag="scores", name="sc")
            flat = PB[h].flatten()
            view = flat[ds(S + ib * P * S, P * S)].rearrange("(p s) -> p s", s=S)
            if accum_shift:
                nc.vector.tensor_copy(sc[:], pc[:])
                nc.gpsimd.dma_start(sc[:], view, accum_op=ADD)
            else:
                sh = blk.tile([P, S], BF16, tag="shift", name="sh")
                if shift_ring == "pool":
                    nc.gpsimd.dma_start(sh[:], view)
                else:
                    nc.sync.dma_start(sh[:], view)
                nc.vector.tensor_tensor(sc[:], pc[:], sh[:], ADD)
            at = blk.tile([P, S], BF16, tag="attn", name="at")
            sums = blk.tile([P, 1], FP32, tag="sums", name="sums")
            nc.scalar.activation(at[:], sc[:], Exp, accum_out=sums[:])
            rec = blk.tile([P, 1], FP32, tag="rec", name="rec")
            nc.vector.reciprocal(rec[:], sums[:])
            nc.vector.tensor_scalar_mul(at[:], at[:], rec[:, 0:1])
            atT = blk.tile([P, NK, P], BF16, tag=f"attnT{h}", name="atT")
            if pe_transpose:
                G = min(4, NK)
                for g in range(NK // G):
                    pt4 = psE.tile([P, G, P], BF16, tag="psE", name="pt4")
                    for j in range(G):
                        kt = G * g + j
                        nc.tensor.transpose(pt4[:, j, :], at[:, ts(kt, P)], ident_sb[:])
                    if g % 2 == 0:
                        nc.vector.tensor_copy(atT[:, ts(g, G), :], pt4[:])
                    else:
                        nc.scalar.copy(atT[:, ts(g, G), :], pt4[:])
            elif fake_transpose:
                nc.sync.dma_start(atT[:], at[:].rearrange("p (a b) -> p a b", b=P))
            else:
                (nc.scalar if transpose_ring == 'act' else nc.sync).dma_start_transpose(atT[:], at[:])
            return atT

        def out_block(ib, atTs):
            o2s = []
            for h in range(2):
                po = psC.tile([D_HEAD, P], FP32, tag="psC", name="po")
                for kt in range(NK):
                    nc.tensor.matmul(
                        po[:],
                        lhsT=vv_sb[:, kt, ds(h * D_HEAD, D_HEAD)],
                        rhs=atTs[h][:, kt, :],
                        start=(kt == 0),
                        stop=(kt == NK - 1),
                    )
                o2 = blk.tile([D_HEAD, P], MMDT, tag=f"o2_{h}", name="o2")
                nc.scalar.copy(o2[:], po[:])
                o2s.append(o2)
            pw = psD.tile([P, D_MODEL], FP32, tag="psD", name="pw")
            for h in range(2):
                nc.tensor.matmul(
                    pw[:],
                    lhsT=o2s[h][:],
                    rhs=Wo_sb[:, h, :],
                    start=(h == 0),
                    stop=(h == 1),
                )
            fin = blk.tile([P, D_MODEL], FP32, tag="fin", name="fin")
            nc.vector.tensor_copy(fin[:], pw[:])
            (nc.scalar if out_ring == 'act' else nc.sync).dma_start(out_partial[ts(ib, P), :], fin[:])

        # ---- main pipeline ----
        def pipeline():
            for h in range(2):
                pos_block(h, 0)
            for ib in range(NB):
                atTs = []
                for h in range(2):
                    if ib + 1 < NB:
                        pos_block(h, ib + 1)
                    atTs.append(head_block(h, ib))
                out_block(ib, atTs)

        if reps == 1:
            pipeline()
        else:
            with tc.For_i(0, reps, 1):
                pipeline()

    nc.finalize()
    return nc


FP8 = mybir.dt.float8e4


def build_nc2(
    S=S_FULL,
    pb_dt=FP8,
    posev="vvvvvvvv",  # evac engine per (chn,h) slot: v=vector, s=scalar
    sh_ring="sp",
    pb_ring="act",
    xbar_ring="sp",
    out_ring="act",
    depth=1,        # out_attnv at ib-depth, out_proj at ib-depth-1
    pos_ahead=2,    # pos pipeline lookahead blocks
    o2_mode="xbar",  # "xbar" | "pe"
):
    """v2: exp-from-PSUM via identity-accum, row-tiled scores, [q,dh] attn@v
    with norm folded into the per-partition o3 evac, merged out-proj,
    software-pipelined pos(+2)/sh(+1)/content(ib)/output(ib-1)."""
    nc = bacc_mod.Bacc()
    NB = S // P
    NK = S // P
    CH = 512
    NCH = S // CH
    CH2 = 1024
    KD = D_MODEL // P

    xT = nc.declare_dram_parameter("xT", [D_MODEL, S], BF16, isOutput=False)
    posT = nc.declare_dram_parameter("posT", [D_MODEL, S], BF16, isOutput=False)
    Wq = nc.declare_dram_parameter("Wq", [D_MODEL, DH2], BF16, isOutput=False)
    Wk = nc.declare_dram_parameter("Wk", [D_MODEL, DH2], BF16, isOutput=False)
    Wv = nc.declare_dram_parameter("Wv", [D_MODEL, DH2], BF16, isOutput=False)
    Wp = nc.declare_dram_parameter("Wp", [D_MODEL, DH2], BF16, isOutput=False)
    Wo128 = nc.declare_dram_parameter("Wo128", [DH2, D_MODEL], BF16, isOutput=False)
    ueff = nc.declare_dram_parameter("ueff", [DH2, 1], FP32, isOutput=False)
    veff = nc.declare_dram_parameter("veff", [DH2, 1], FP32, isOutput=False)
    out_partial = nc.declare_dram_parameter("out_partial", [S, D_MODEL], BF16, isOutput=True)

    def ring(which):
        return nc.scalar if which == "act" else nc.sync

    with ExitStack() as ctx:
        tc = ctx.enter_context(tile.TileContext(nc))
        consts = ctx.enter_context(tc.tile_pool(name="consts", bufs=1))
        blk = ctx.enter_context(tc.tile_pool(name="blk", bufs=2))
        dram = ctx.enter_context(tc.tile_pool(name="dram", bufs=1, space="DRAM"))
        ps = ctx.enter_context(tc.tile_pool(name="ps", bufs=1, space="PSUM"))

        # ---- load constants / inputs ----
        xT_sb = consts.tile([P, KD, S], BF16)
        nc.sync.dma_start(xT_sb[:], xT.rearrange("(o p) s -> p o s", p=P))
        posT_sb = consts.tile([P, KD, S], BF16)
        nc.sync.dma_start(posT_sb[:], posT.rearrange("(o p) s -> p o s", p=P))
        w_sbs = {}
        for nm, handle in (("Wq", Wq), ("Wk", Wk), ("Wv", Wv), ("Wp", Wp)):
            w_sb = consts.tile([P, KD, DH2], BF16, name=f"{nm}_sb")
            nc.sync.dma_start(w_sb[:], handle.rearrange("(o p) m -> p o m", p=P))
            w_sbs[nm] = w_sb
        Wo_sb = consts.tile([DH2, D_MODEL], BF16)
        nc.sync.dma_start(Wo_sb[:], Wo128[:, :])
        identsh = consts.tile([P, P], pb_dt)
        make_identity(nc, identsh[:])
        identb = consts.tile([P, P], BF16)
        make_identity(nc, identb[:])
        ueff_sb = consts.tile([DH2, 1], FP32)
        nc.sync.dma_start(ueff_sb[:], ueff[:, :])
        veff_sb = consts.tile([DH2, 1], FP32)
        nc.sync.dma_start(veff_sb[:], veff[:, :])

        qTu = consts.tile([DH2, S], BF16)
        qTv = consts.tile([DH2, S], BF16)
        kT = consts.tile([DH2, S], BF16)
        pT = consts.tile([DH2, S], BF16)

        # ---- projections (reuse steady-state PSUM tags) ----
        def proj_T(w_sb, src_sb, dsts):
            for half in range(S // CH2):
                pq = ps.tile([P, CH2], FP32, tag="pc", bufs=2, name="pq")
                for seg in range(CH2 // CH):
                    for kt in range(KD):
                        nc.tensor.matmul(
                            pq[:, ts(seg, CH)],
                            lhsT=w_sb[:, kt, :],
                            rhs=src_sb[:, kt, ds(half * CH2 + seg * CH, CH)],
                            start=(kt == 0),
                            stop=(kt == KD - 1),
                        )
                for dst, scale, bias in dsts:
                    if scale is None:
                        nc.scalar.copy(dst[:, ts(half, CH2)], pq[:])
                    else:
                        nc.vector.tensor_scalar(
                            dst[:, ts(half, CH2)], pq[:], scale, bias, MULT, ADD
                        )

        proj_T(w_sbs["Wq"], xT_sb, [(qTu, ISQ, ueff_sb[:, 0:1]), (qTv, ISQ, veff_sb[:, 0:1])])
        proj_T(w_sbs["Wk"], xT_sb, [(kT, None, None)])
        proj_T(w_sbs["Wp"], posT_sb, [(pT, None, None)])

        vv_sb = consts.tile([P, NK, DH2], BF16)
        for st in range(NK):
            pv = ps.tile([P, DH2], FP32, tag=("po" if st % 2 == 0 else "pw"), bufs=1, name="pv")
            for kt in range(KD):
                nc.tensor.matmul(
                    pv[:],
                    lhsT=xT_sb[:, kt, ts(st, P)],
                    rhs=w_sbs["Wv"][:, kt, :],
                    start=(kt == 0),
                    stop=(kt == KD - 1),
                )
            nc.any.tensor_copy(vv_sb[:, st, :], pv[:])

        # ---- per-head DRAM pos-score buffers, rows [0 | posrow] ----
        PB = [dram.tile([S, S + 1], pb_dt, name=f"pb{h}") for h in range(2)]

        def new_frags():
            fr = []
            for h in range(2):
                f = blk.tile([P, S + 1], pb_dt, tag=f"frag{h}", name=f"frag{h}")
                nc.gpsimd.memset(f[:, 0:1], 0.0)
                fr.append(f)
            return fr

        def pos_chunks(ib, chns, frags):
            for chn in chns:
                for h in range(2):
                    pp = ps.tile([P, CH], FP32, tag="pp", bufs=2, name="pp")
                    nc.tensor.matmul(
                        pp[:],
                        lhsT=qTv[ds(h * D_HEAD, D_HEAD), ts(ib, P)],
                        rhs=pT[ds(h * D_HEAD, D_HEAD), ts(chn, CH)],
                        start=True,
                        stop=True,
                    )
                    dst = frags[h][:, 1 + chn * CH : 1 + (chn + 1) * CH]
                    if posev[chn * 2 + h] == "v":
                        nc.vector.tensor_copy(dst, pp[:])
                    else:
                        nc.scalar.copy(dst, pp[:])

        def pb_write(ib, frags):
            for h in range(2):
                ring(pb_ring).dma_start(PB[h][ts(ib, P), :], frags[h][:])

        def sh_read(ib):
            out = []
            for h in range(2):
                shh = blk.tile([P, S], pb_dt, tag=f"sh{h}", name=f"sh{h}")
                flat = PB[h].flatten()
                view = flat[ds(S + ib * P * S, P * S)].rearrange("(p s) -> p s", s=S)
                for half in range(2):
                    cols = ds(half * (S // 2), S // 2)
                    ring(sh_ring).dma_start(shh[:, cols], view[:, cols])
                out.append(shh)
            return out

        def attnv_part(po, atT2, kts):
            # single PSUM bank, ONE start group: only the very first MM has
            # start=True (a start resets the whole bank)
            for kt in kts:
                for h in range(2):
                    nc.tensor.matmul(
                        po[:, ds(h * D_HEAD, D_HEAD)],
                        lhsT=atT2[:, kt // (CH2 // P), h * (NK // 2) + (kt % (CH2 // P)), :],
                        rhs=vv_sb[:, kt, ds(h * D_HEAD, D_HEAD)],
                        start=(kt == 0 and h == 0),
                        stop=(kt == NK - 1 and h == 1),
                        skip_group_check=True,
                    )

        def attnv_finish(po, recs):
            o3 = blk.tile([P, DH2], BF16, tag="o3", name="o3")
            for h in range(2):
                nc.vector.tensor_scalar_mul(
                    o3[:, ds(h * D_HEAD, D_HEAD)],
                    po[:, ds(h * D_HEAD, D_HEAD)],
                    recs[h][:, 0:1],
                )
            o2 = blk.tile([DH2, P], BF16, tag="o2", bufs=2, name="o2")
            if o2_mode == "xbar":
                ring(xbar_ring).dma_start_transpose(o2[:], o3[:])
            else:
                pt = ps.tile([DH2, P], BF16, tag="po", name="pt")
                nc.tensor.transpose(pt[:], o3[:], identb[:])
                nc.scalar.copy(o2[:], pt[:])
            return o2

        def out_proj(ib, o2):
            pw = ps.tile([P, D_MODEL], FP32, tag="pw", name="pw")
            nc.tensor.matmul(pw[:], lhsT=o2[:], rhs=Wo_sb[:], start=True, stop=True)
            fin = blk.tile([P, D_MODEL], BF16, tag="fin", name="fin")
            nc.vector.tensor_copy(fin[:], pw[:])
            ring(out_ring).dma_start(out_partial[ts(ib, P), :], fin[:])

        # ---- software-pipelined main loop ----
        for pib in range(pos_ahead):
            fr = new_frags()
            pos_chunks(pib, range(NCH), fr)
            pb_write(pib, fr)
        sh_cur = sh_read(0)
        prev = None   # (ib, atT2, recs) awaiting attnv
        o2q = []      # (ib, o2) awaiting out_proj
        for ib in range(NB):
            sh_next = sh_read(ib + 1) if (ib + 1 < NB and pos_ahead > 2) else None
            fr = new_frags() if ib + pos_ahead < NB else None
            po_prev = ps.tile([P, DH2], FP32, tag="po", name="po") if prev is not None else None
            sums2 = []
            at2 = blk.tile([P, 2, 2, CH2], BF16, tag="at2", bufs=3, name="at2")
            atT2 = blk.tile([P, 2, NK, P], BF16, tag="atT2", bufs=3, name="atT2")
            for h in range(2):
                sums2.append(blk.tile([P, 2], FP32, tag=f"sums{h}", bufs=3, name=f"sums{h}"))
            for half in range(S // CH2):
                pcs = [
                    ps.tile([P, CH2], FP32, tag="pc", bufs=2, name=f"pc{h}")
                    for h in range(2)
                ]
                for seg in range(CH2 // CH):
                    cols = ds(half * CH2 + seg * CH, CH)
                    for h in range(2):
                        nc.tensor.matmul(
                            pcs[h][:, ts(seg, CH)],
                            lhsT=qTu[ds(h * D_HEAD, D_HEAD), ts(ib, P)],
                            rhs=kT[ds(h * D_HEAD, D_HEAD), cols],
                            start=True,
                            stop=False,
                        )
                    for h in range(2):
                        nc.tensor.matmul(
                            pcs[h][:, ts(seg, CH)],
                            lhsT=identsh[:],
                            rhs=sh_cur[h][:, cols],
                            start=False,
                            stop=True,
                            skip_group_check=True,
                        )
                if fr is not None:
                    pos_chunks(ib + pos_ahead, (2 * half, 2 * half + 1), fr)
                for h in range(2):
                    nc.scalar.activation(
                        at2[:, half, h, :],
                        pcs[h][:],
                        Exp,
                        accum_out=sums2[h][:, ds(half, 1)],
                    )
                ring(xbar_ring).dma_start_transpose(
                    atT2[:, half, :, :], at2[:, half, :, :]
                )
                if po_prev is not None:
                    attnv_part(
                        po_prev, prev[1],
                        range(half * (NK // 2), (half + 1) * (NK // 2)),
                    )
            recs = []
            for h in range(2):
                stot = blk.tile([P, 1], FP32, tag=f"stot{h}", name=f"stot{h}")
                nc.vector.tensor_tensor(stot[:], sums2[h][:, 0:1], sums2[h][:, 1:2], ADD)
                rec = blk.tile([P, 1], FP32, tag=f"rec{h}", bufs=3, name=f"rec{h}")
                nc.vector.reciprocal(rec[:], stot[:])
                recs.append(rec)
            if fr is not None:
                pb_write(ib + pos_ahead, fr)
            if pos_ahead > 2:
                if sh_next is not None:
                    sh_cur = sh_next
            elif ib + 1 < NB:
                sh_cur = sh_read(ib + 1)
            if po_prev is not None:
                o2q.append((prev[0], attnv_finish(po_prev, prev[2])))
            if len(o2q) > 1:
                j, o = o2q.pop(0)
                out_proj(j, o)
            prev = (ib, atT2, recs)
        po_last = ps.tile([P, DH2], FP32, tag="po", name="po")
        attnv_part(po_last, prev[1], range(NK))
        o2q.append((prev[0], attnv_finish(po_last, prev[2])))
        for j, o in o2q:
            out_proj(j, o)

    nc.finalize()
    return nc


# ---------------- host side ----------------

USE_V2 = True

_NC_CACHE = {}


def _get_nc(S=S_FULL):
    key = (S, USE_V2)
    if key not in _NC_CACHE:
        _NC_CACHE[key] = build_nc2(S) if USE_V2 else build_nc(S)
    return _NC_CACHE[key]


def make_in_maps(inputs, S=S_FULL, n_cores=8):
    x = np.ascontiguousarray(np.asarray(inputs["x"], np.float32))
    pos = np.ascontiguousarray(np.asarray(inputs["pos_embedding"], np.float32))
    Wq = np.asarray(inputs["Wq"], np.float32)
    bq = np.asarray(inputs["bq"], np.float32)
    Wk = np.asarray(inputs["Wk"], np.float32)
    Wv = np.asarray(inputs["Wv"], np.float32)
    Wp = np.asarray(inputs["Wp"], np.float32)
    u = np.asarray(inputs["u"], np.float32)
    v = np.asarray(inputs["v"], np.float32)
    Wo = np.asarray(inputs["Wo"], np.float32)

    in_maps = []
    for c in range(n_cores):
        b = c // 4
        h0 = 2 * (c % 4)
        sl = slice(h0 * D_HEAD, (h0 + 2) * D_HEAD)
        u_eff = ((u[h0 : h0 + 2].reshape(-1) + bq[sl]) * ISQ).astype(np.float32)
        v_eff = ((v[h0 : h0 + 2].reshape(-1) + bq[sl]) * ISQ).astype(np.float32)
        m = {
            "xT": np.ascontiguousarray(x[b, :S].T).astype(BF16NP),
            "posT": np.ascontiguousarray(pos[b, :S].T).astype(BF16NP),
            "Wq": np.ascontiguousarray(Wq[:, sl]).astype(BF16NP),
            "Wk": np.ascontiguousarray(Wk[:, sl]).astype(BF16NP),
            "Wv": np.ascontiguousarray(Wv[:, sl]).astype(BF16NP),
            "Wp": np.ascontiguousarray(Wp[:, sl]).astype(BF16NP),
            "ueff": u_eff.reshape(DH2, 1),
            "veff": v_eff.reshape(DH2, 1),
        }
        if USE_V2:
            m["Wo128"] = np.ascontiguousarray(Wo[sl, :]).astype(BF16NP)
        else:
            m["Wo2"] = np.ascontiguousarray(
                Wo[sl, :].reshape(2, D_HEAD, D_MODEL)
            ).astype(BF16NP)
        in_maps.append(m)
    return in_maps


def assemble(inputs, results, S=S_FULL):
    bv = np.asarray(inputs["bv"], np.float64)
    Wo = np.asarray(inputs["Wo"], np.float64)
    bo = np.asarray(inputs["bo"], np.float64)
    const = (bv @ Wo + bo).astype(np.float32)
    out = np.zeros((B_FULL, S, D_MODEL), np.float32)
    for c, res in enumerate(results):
        out[c // 4] += np.asarray(res["out_partial"], np.float32)
    out += const[None, None, :]
    return out


def _run(inputs, trace=False, **kw):
    nc = _get_nc(S_FULL)
    in_maps = make_in_maps(inputs, S_FULL)
    res = run_bass_kernel_spmd(nc, in_maps, list(range(8)), trace=trace, **kw)
    out = assemble(inputs, res.results, S_FULL)
    return out, res


def kernel(**inputs) -> np.ndarray:
    out, _ = _run(inputs, trace=False)
    return out

